# revision 1
# baseline (speedup 1.0000x reference)
"""Causal multi-head attention block (QKV proj + flash-style attention + out proj)
for Trainium2, sharded over 8 NeuronCores as (batch, head-group):
core c -> batch b = c//2, heads hg*4..hg*4+4 with hg = c%2.

Each core computes, for its batch and its 4 heads:
  QKV projection (bf16 matmuls, fp32 PSUM)
  S^T = K @ Q^T per (128k x 512q) tile, causal-pruned
  P = exp(SCALE * S^T)  (no max subtraction: scores are O(1) by construction)
  O^T = V^T-chunks @ P  accumulated over k-tiles, l = ones^T @ P (row sums)
  O^T_norm = O^T * broadcast(1/l)
  partial out = sum_h O_h^T.T @ Wproj_h (+ bias on even cores)
Host sums the two per-batch partials to unshard.
"""

import numpy as np
import ml_dtypes

import concourse.bass as bass
import concourse.tile as tile
from concourse import bacc, mybir
from concourse.bass_utils import run_bass_kernel_spmd

B, N, C, H = 4, 2048, 256, 8
SCALE = C ** -0.5
BF16 = ml_dtypes.bfloat16
FP32 = mybir.dt.float32
BF = mybir.dt.bfloat16
HPC = 4  # heads per core


def _emit(tc, nq, aps):
    nc = tc.nc
    nb = nq // 512   # 512-wide n/q blocks
    nt = nq // 128   # 128-wide n chunks

    xt_d, wqkv_d, wproj_d, bias_d, mask_d, ones_d, out_d = aps
    xt_r = xt_d.rearrange("(c p) n -> p c n", p=128)
    wqkv_r = wqkv_d.rearrange("(c p) m -> p c m", p=128)
    wproj_r = wproj_d.rearrange("(t p) f -> p t f", p=128)
    out_r = out_d.rearrange("(t p) f -> p t f", p=128)

    singles = tc._es.enter_context(tc.tile_pool(name="singles", bufs=1))
    pool_qkv = tc._es.enter_context(tc.tile_pool(name="qkvp", bufs=2))
    pool_p = tc._es.enter_context(tc.tile_pool(name="pp", bufs=6))
    pool_misc = tc._es.enter_context(tc.tile_pool(name="miscp", bufs=2))
    pool_osb = tc._es.enter_context(tc.tile_pool(name="osbp", bufs=6))
    pool_ot = tc._es.enter_context(tc.tile_pool(name="psumot", bufs=2, space="PSUM"))
    pool_s = tc._es.enter_context(tc.tile_pool(name="psums", bufs=3, space="PSUM"))
    pool_l = tc._es.enter_context(tc.tile_pool(name="psuml", bufs=1, space="PSUM"))

    # --- load constants / inputs ---
    xt_sb = singles.tile([128, 2, nq], BF)
    wqkv_sb = singles.tile([128, 2, 3 * HPC * C], BF)
    wproj_sb = singles.tile([128, 2 * HPC, C], BF)
    bias_sb = singles.tile([1, C], FP32)
    biasb_sb = singles.tile([128, C], FP32)
    mask_sb = singles.tile([128, 128], BF)
    ones_sb = singles.tile([128, 1], BF)
    ot_sb = singles.tile([128, 2 * HPC, nq], BF)

    # split the critical input DMAs into chunks, spread across BOTH HW-DGE
    # rings (sync + scalar issue different rings) so the first QKV matmuls
    # can start as early as possible
    for ib in range(nb):
        nc.sync.dma_start(xt_sb[:, :, ib * 512:(ib + 1) * 512],
                          xt_r[:, :, ib * 512:(ib + 1) * 512])
    for hw in range(HPC):
        c0 = hw * 3 * C
        nc.scalar.dma_start(wqkv_sb[:, :, c0:c0 + 3 * C], wqkv_r[:, :, c0:c0 + 3 * C])
    nc.sync.dma_start(mask_sb[:], mask_d[:])
    nc.sync.dma_start(ones_sb[:], ones_d[:])
    nc.scalar.dma_start(wproj_sb[:], wproj_r)
    nc.scalar.dma_start(bias_sb[:], bias_d[:])
    nc.gpsimd.partition_broadcast(biasb_sb[:], bias_sb[:])

    # warm up the PE HAM clock gate with dummy matmuls while input DMAs land
    warm_sb = singles.tile([128, 512], BF)
    nc.gpsimd.memset(warm_sb[:], 0.0)
    warm_ps = pool_ot.tile([128, 512], FP32, tag="ot0", name="warm_ps")
    for wi in range(30):
        nc.tensor.matmul(warm_ps[:], warm_sb[:, :128], warm_sb[:],
                         start=(wi == 0), stop=(wi == 29))

    s_ring = [pool_s.tile([128, 512], FP32, tag="s", name=f"sring{i}")
              for i in range(3)]
    p_ring = [pool_p.tile([128, 512], BF, tag="p", name=f"pring{i}")
              for i in range(6)]
    ring = {"i": 0}

    def alloc_head_tiles():
        qt_sb = pool_qkv.tile([128, 2, nq], BF, tag="qt", name="qt")
        kt_sb = pool_qkv.tile([128, 2, nq], BF, tag="kt", name="kt")
        v_sb = pool_qkv.tile([128, nt, C], BF, tag="v", name="v")
        return qt_sb, kt_sb, v_sb

    def qkv_blocks(hp, tiles):
        """One closure per (128,512) projection block of head hp."""
        qt_sb, kt_sb, v_sb = tiles
        blocks = []

        def qk_block(j, ct, ib, tgt, par):
            def go():
                ri = ring["i"]; ring["i"] += 1
                ps = s_ring[ri % 3]
                col0 = (hp * 3 + j) * C + ct * 128
                for ci in range(2):
                    nc.tensor.matmul(
                        ps[:],
                        wqkv_sb[:, ci, col0:col0 + 128],
                        xt_sb[:, ci, ib * 512:(ib + 1) * 512],
                        start=(ci == 0), stop=(ci == 1),
                    )
                if par % 2 == 0:
                    nc.scalar.copy(tgt[:, ct, ib * 512:(ib + 1) * 512], ps[:])
                else:
                    nc.vector.tensor_copy(tgt[:, ct, ib * 512:(ib + 1) * 512], ps[:])
            return go

        def v_block(it, par):
            def go():
                ri = ring["i"]; ring["i"] += 1
                ps = s_ring[ri % 3]
                vcol = (hp * 3 + 2) * C
                for ci in range(2):
                    nc.tensor.matmul(
                        ps[:, :C],
                        xt_sb[:, ci, it * 128:(it + 1) * 128],
                        wqkv_sb[:, ci, vcol:vcol + C],
                        start=(ci == 0), stop=(ci == 1),
                    )
                if par % 2 == 0:
                    nc.scalar.copy(v_sb[:, it, :], ps[:, :C])
                else:
                    nc.vector.tensor_copy(v_sb[:, it, :], ps[:, :C])
            return go

        par = 0
        for j, ti in ((0, 0), (1, 1)):
            for ct in range(2):
                for ib in range(nb):
                    blocks.append(qk_block(j, ct, ib, tiles[ti], par))
                    par += 1
        for it in range(nt):
            blocks.append(v_block(it, par))
            par += 1
        return blocks

    def attention(hp, tiles, next_blocks):
        """Flash attention for head hp; next head's QKV blocks are drip-fed
        into the PE stream to absorb per-cycle semaphore bubbles."""
        qt_sb, kt_sb, v_sb = tiles
        steps = []
        for qb in range(nb):
            kmax = 4 * qb + 4
            for kt in range(kmax):
                q_off = max(0, kt * 128 - qb * 512)
                steps.append((qb, kt, q_off, 512 - q_off, kt == 0, kt == kmax - 1))

        state = {}

        def emit_S(step):
            qb, kt, q_off, nqf, first, last = step
            if first:
                state[qb] = (
                    pool_ot.tile([128, 512], FP32, tag="ot0", name="ot0"),
                    pool_ot.tile([128, 512], FP32, tag="ot1", name="ot1"),
                    pool_l.tile([1, 512], FP32, tag="l", name="lp"),
                )
            q0 = qb * 512 + q_off
            ri = ring["i"]; ring["i"] += 1
            s_ps = s_ring[ri % 3]
            for ci in range(2):
                nc.tensor.matmul(
                    s_ps[:, :nqf],
                    kt_sb[:, ci, kt * 128:(kt + 1) * 128],
                    qt_sb[:, ci, q0:q0 + nqf],
                    start=(ci == 0), stop=(ci == 1),
                )
            p_sb = p_ring[ri % 6]
            nc.scalar.activation(
                p_sb[:, :nqf], s_ps[:, :nqf],
                mybir.ActivationFunctionType.Exp, scale=SCALE,
            )
            if kt >= 4 * qb:  # diagonal tile: causal mask on first 128 cols
                nc.vector.tensor_tensor(
                    p_sb[:, :128], p_sb[:, :128], mask_sb[:], mybir.AluOpType.mult
                )
            return p_sb

        def emit_PV(step, p_sb):
            qb, kt, q_off, nqf, first, last = step
            ot0, ot1, lp = state[qb]
            nc.tensor.matmul(ot1[:, q_off:], v_sb[:, kt, 0:128], p_sb[:, :nqf],
                             start=first, stop=last)
            nc.tensor.matmul(ot0[:, q_off:], v_sb[:, kt, 128:256], p_sb[:, :nqf],
                             start=first, stop=last)

        def emit_L(step, p_sb):
            qb, kt, q_off, nqf, first, last = step
            ot0, ot1, lp = state[qb]
            nc.tensor.matmul(lp[:, q_off:], ones_sb[:], p_sb[:, :nqf],
                             start=first, stop=last)
            if last:
                rl_sb = pool_misc.tile([1, 512], FP32, tag="rl", name="rl")
                rb_sb = pool_misc.tile([128, 512], FP32, tag="rb", name="rb")
                nc.vector.reciprocal_approx_fast(rl_sb[:], lp[:])
                nc.gpsimd.partition_broadcast(rb_sb[:], rl_sb[:])
                for ct, otp in ((0, ot1), (1, ot0)):
                    nc.vector.tensor_tensor(
                        ot_sb[:, hp * 2 + ct, qb * 512:(qb + 1) * 512],
                        otp[:], rb_sb[:], mybir.AluOpType.mult,
                    )

        # software pipeline; PE emission order per slot is
        #   S(i), L(i-2), PV(i-1), [next head's QKV block]
        work = []
        emitted = 0
        for i, step in enumerate(steps):
            work.append((step, emit_S(step)))
            if i >= 2:
                emit_L(*work[i - 2])
            if i >= 1:
                emit_PV(*work[i - 1])
            want = (i + 1) * len(next_blocks) // len(steps)
            while emitted < want:
                next_blocks[emitted]()
                emitted += 1
        n = len(steps)
        if n >= 2:
            emit_L(*work[n - 2])
        emit_PV(*work[n - 1])
        emit_L(*work[n - 1])
        while emitted < len(next_blocks):
            next_blocks[emitted]()
            emitted += 1

    head_tiles = alloc_head_tiles()
    for b in qkv_blocks(0, head_tiles):
        b()
    for hp in range(HPC):
        if hp + 1 < HPC:
            nxt_tiles = alloc_head_tiles()
            nxt = qkv_blocks(hp + 1, nxt_tiles)
        else:
            nxt_tiles, nxt = None, []
        attention(hp, head_tiles, nxt)
        head_tiles = nxt_tiles

    # --- output projection, all heads accumulated in PSUM ---
    # round-robin psum across all three tag families (attention pools are
    # done by now) for more outstanding chunks
    prj_pools = [(pool_s, "s"), (pool_ot, "ot0"), (pool_ot, "ot1")]
    for it in range(nt):
        pp, ptag = prj_pools[it % 3]
        ps = pp.tile([128, 512], FP32, tag=ptag, name="ps_prj")
        for t in range(2 * HPC):
            nc.tensor.matmul(
                ps[:, :C],
                ot_sb[:, t, it * 128:(it + 1) * 128],
                wproj_sb[:, t, :],
                start=(t == 0), stop=(t == 2 * HPC - 1),
            )
        osb = pool_osb.tile([128, C], FP32, tag="osb", name="osb")
        nc.vector.tensor_tensor(osb[:], ps[:, :C], biasb_sb[:], mybir.AluOpType.add)
        nc.sync.dma_start(out_r[:, it, :], osb[:])


def build_program(nq=N):
    nc = bacc.Bacc(trn_type="TRN2")
    xt_d = nc.dram_tensor("xt", (C, nq), BF, kind="ExternalInput").ap()
    wqkv_d = nc.dram_tensor("wqkv", (C, 3 * HPC * C), BF, kind="ExternalInput").ap()
    wproj_d = nc.dram_tensor("wproj", (2 * HPC * 128, C), BF, kind="ExternalInput").ap()
    bias_d = nc.dram_tensor("bias", (1, C), FP32, kind="ExternalInput").ap()
    mask_d = nc.dram_tensor("mask", (128, 128), BF, kind="ExternalInput").ap()
    ones_d = nc.dram_tensor("ones", (128, 1), BF, kind="ExternalInput").ap()
    out_d = nc.dram_tensor("out", (nq, C), FP32, kind="ExternalOutput").ap()
    with tile.TileContext(nc) as tc:
        import contextlib
        tc._es = contextlib.ExitStack()
        with tc._es:
            _emit(tc, nq, (xt_d, wqkv_d, wproj_d, bias_d, mask_d, ones_d, out_d))
    nc.compile()
    return nc


def core_inputs(core, x, w_qkv, w_proj, b_proj, nq=N):
    b, hg = core // 2, core % 2
    heads = list(range(hg * HPC, hg * HPC + HPC))
    xt = np.ascontiguousarray(x[b].T).astype(BF16)
    wr = np.asarray(w_qkv, np.float32).reshape(C, 3, H, C)
    w4 = np.ascontiguousarray(
        wr[:, :, heads, :].transpose(0, 2, 1, 3)
    ).reshape(C, 3 * HPC * C).astype(BF16)
    wp = np.asarray(w_proj, np.float32).reshape(H, C, C)[heads].reshape(HPC * C, C).astype(BF16)
    bias = (np.asarray(b_proj, np.float32) if hg == 0
            else np.zeros(C, np.float32)).reshape(1, C)
    mask = (np.arange(128)[:, None] <= np.arange(128)[None, :]).astype(BF16)
    ones = np.ones((128, 1), BF16)
    return {"xt": xt, "wqkv": w4, "wproj": wp, "bias": bias,
            "mask": mask, "ones": ones}


_CACHE = {}


def kernel(x, w_qkv, w_proj, b_proj, **run_kwargs):
    x = np.asarray(x, np.float32)
    w_qkv = np.asarray(w_qkv, np.float32)
    w_proj = np.asarray(w_proj, np.float32)
    b_proj = np.asarray(b_proj, np.float32)
    if "nc" not in _CACHE:
        _CACHE["nc"] = build_program(N)
    nc = _CACHE["nc"]
    in_maps = [core_inputs(c, x, w_qkv, w_proj, b_proj) for c in range(8)]
    res = run_bass_kernel_spmd(nc, in_maps, core_ids=list(range(8)), **run_kwargs)
    out = np.zeros((B, N, C), np.float32)
    for c in range(8):
        out[c // 2] += res.results[c]["out"]
    _CACHE["last_results"] = res
    return out



# revision 2
# speedup vs baseline: 1.1844x; 1.1844x over previous
"""Causal multi-head attention block (QKV proj + flash-style attention + out proj)
for Trainium2, sharded over 8 NeuronCores as (batch, head-group):
core c -> batch b = c//2, heads hg*4..hg*4+4 with hg = c%2.

Row-split precision hybrid per 512-wide q-block:
  qb0 (rows 0:512, concentrated softmax): bf16 everywhere (error-sensitive).
  qb1-3 (rows 512:2048): fp8(e4m3) DoubleRow matmuls for S, P@V and the
    row-sum L; exp is computed as exp(S*SCALE - SHIFT) so P <= ~60 stays
    inside TRN e4m3 range (+-240); the shift cancels in O/l exactly.
  Row sums l are computed with an all-ones [128,k] stationary so the PE
  broadcasts l to all 128 partitions (no gpsimd partition-broadcast needed).
  Out projection runs transposed (w_proj stationary, O^T moving) for
  stationary reuse; host transposes the (C, N) result back.
"""

import numpy as np
import ml_dtypes

import concourse.bass as bass
import concourse.tile as tile
from concourse import bacc, mybir
from concourse.bass_utils import run_bass_kernel_spmd

B, N, C, H = 4, 2048, 256, 8
SCALE = C ** -0.5
SHIFT = 2.0
BF16 = ml_dtypes.bfloat16
FP32 = mybir.dt.float32
BF = mybir.dt.bfloat16
F8 = mybir.dt.float8e4
HPC = 4  # heads per core
DR = mybir.MatmulPerfMode.DoubleRow


def _emit(tc, nq, aps):
    nc = tc.nc
    nt = nq // 128

    xt_d, wqkv_d, wproj_d, bias_d, mask_d, ztri_d, out_d = aps
    xt_r = xt_d.rearrange("(c p) n -> p c n", p=128)
    wqkv_r = wqkv_d.rearrange("(c p) m -> p c m", p=128)
    wproj_r = wproj_d.rearrange("(t p) f -> p t f", p=128)
    out_r = out_d.rearrange("(t p) n -> p t n", p=128)

    singles = tc._es.enter_context(tc.tile_pool(name="singles", bufs=1))
    pool_head = tc._es.enter_context(tc.tile_pool(name="headp", bufs=2))
    pool_p16 = tc._es.enter_context(tc.tile_pool(name="p16p", bufs=3))
    pool_p8 = tc._es.enter_context(tc.tile_pool(name="p8p", bufs=3))
    pool_rb = tc._es.enter_context(tc.tile_pool(name="rbp", bufs=2))
    pool_osb = tc._es.enter_context(tc.tile_pool(name="osbp", bufs=4))
    # PSUM: sp 2x2 banks + ot0/ot1 2 + l 1 + qp 1 = 8 banks
    pool_sp = tc._es.enter_context(tc.tile_pool(name="psumsp", bufs=2, space="PSUM"))
    pool_ot = tc._es.enter_context(tc.tile_pool(name="psumot", bufs=1, space="PSUM"))
    pool_l = tc._es.enter_context(tc.tile_pool(name="psuml", bufs=1, space="PSUM"))
    pool_qp = tc._es.enter_context(tc.tile_pool(name="psumqp", bufs=1, space="PSUM"))

    # --- SBUF constants / inputs ---
    xt_sb = singles.tile([128, 2, nq], BF)
    wqkv_sb = singles.tile([128, 2, 3 * HPC * C], BF)
    wproj_sb = singles.tile([128, 2 * HPC, C], BF)
    bias_sb = singles.tile([128, 2], FP32)
    mask_sb = singles.tile([128, 128], BF)
    ztri_sb = singles.tile([128, 256], BF)
    tri8_sb = singles.tile([128, 128], F8)
    ztri8_sb = singles.tile([128, 256], F8)
    ones16_sb = singles.tile([128, 128], BF)
    ones8_sb = singles.tile([128, 2, 128], F8)
    shiftb_sb = singles.tile([128, 1], FP32)
    ot_sb = singles.tile([128, 2 * HPC, nq], BF)

    # input DMAs split across both HW-DGE rings
    for ib in range(4):
        nc.sync.dma_start(xt_sb[:, :, ib * 512:(ib + 1) * 512],
                          xt_r[:, :, ib * 512:(ib + 1) * 512])
    for hw in range(HPC):
        c0 = hw * 3 * C
        nc.scalar.dma_start(wqkv_sb[:, :, c0:c0 + 3 * C], wqkv_r[:, :, c0:c0 + 3 * C])
    nc.sync.dma_start(mask_sb[:], mask_d[:])
    nc.sync.dma_start(ztri_sb[:], ztri_d[:])
    nc.scalar.dma_start(wproj_sb[:], wproj_r)
    nc.scalar.dma_start(bias_sb[:], bias_d[:])
    nc.vector.tensor_copy(tri8_sb[:], mask_sb[:])
    nc.vector.tensor_copy(ztri8_sb[:], ztri_sb[:])
    nc.gpsimd.memset(ones16_sb[:], 1.0)
    nc.gpsimd.memset(ones8_sb[:], 1.0)
    nc.gpsimd.memset(shiftb_sb[:], -SHIFT)

    # warm up the PE HAM clock gate while input DMAs land
    warm_sb = singles.tile([128, 512], BF)
    nc.gpsimd.memset(warm_sb[:], 0.0)
    warm_ps = pool_qp.tile([128, 512], FP32, tag="qp", name="warm_ps")
    for wi in range(30):
        nc.tensor.matmul(warm_ps[:], warm_sb[:, :128], warm_sb[:],
                         start=(wi == 0), stop=(wi == 29))

    def alloc_head_tiles():
        qt16 = pool_head.tile([128, 2, 512], BF, tag="qt16", name="qt16")
        kt16 = pool_head.tile([128, 2, 512], BF, tag="kt16", name="kt16")
        qt8 = pool_head.tile([128, 2, nq], F8, tag="qt8", name="qt8")
        kt8 = pool_head.tile([128, 2, nq], F8, tag="kt8", name="kt8")
        v16 = pool_head.tile([128, 4, C], BF, tag="v16", name="v16")
        v8 = pool_head.tile([128, nt, C], F8, tag="v8", name="v8")
        return qt16, kt16, qt8, kt8, v16, v8

    par = {"i": 0}

    def cast(dst, src):
        """PSUM -> SBUF cast, alternating between ACT and DVE engines."""
        par["i"] += 1
        if par["i"] % 2 == 0:
            nc.scalar.copy(dst, src)
        else:
            nc.vector.tensor_copy(dst, src)

    def qkv_blocks(hp, tiles, psum_alloc):
        """Per-(128x512)-block closures for head hp's QKV projection.
        psum_alloc() -> a [128, 512] fp32 PSUM tile."""
        qt16, kt16, qt8, kt8, v16, v8 = tiles
        blocks = []

        def qk_block(j, ct, ib):
            def go():
                ps = psum_alloc()
                col0 = (hp * 3 + j) * C + ct * 128
                for ci in range(2):
                    nc.tensor.matmul(
                        ps[:], wqkv_sb[:, ci, col0:col0 + 128],
                        xt_sb[:, ci, ib * 512:(ib + 1) * 512],
                        start=(ci == 0), stop=(ci == 1),
                    )
                tgt8 = qt8 if j == 0 else kt8
                if ib == 0:
                    tgt16 = qt16 if j == 0 else kt16
                    cast(tgt16[:, ct, :], ps[:])
                    if j == 1:
                        cast(tgt8[:, ct, 0:512], ps[:])
                else:
                    cast(tgt8[:, ct, ib * 512:(ib + 1) * 512], ps[:])
            return go

        def v_block(it):
            def go():
                ps = psum_alloc()
                vcol = (hp * 3 + 2) * C
                for ci in range(2):
                    nc.tensor.matmul(
                        ps[:, :C], xt_sb[:, ci, it * 128:(it + 1) * 128],
                        wqkv_sb[:, ci, vcol:vcol + C],
                        start=(ci == 0), stop=(ci == 1),
                    )
                cast(v8[:, it, :], ps[:, :C])
                if it < 4:
                    cast(v16[:, it, :], ps[:, :C])
            return go

        # order: qb0-critical first (q/k ib0, v it0-3), then fp8 operands
        for j in (0, 1):
            for ct in range(2):
                blocks.append(qk_block(j, ct, 0))
        for it in range(4):
            blocks.append(v_block(it))
        for j in (1, 0):
            for ct in range(2):
                for ib in range(1, 4):
                    blocks.append(qk_block(j, ct, ib))
        for it in range(4, nt):
            blocks.append(v_block(it))
        return blocks

    def attention(hp, tiles, next_blocks):
        """Flash attention for head hp: qb0 in bf16, qb1-3 in fp8 DoubleRow.
        next head's QKV blocks are drip-fed into the PE stream."""
        qt16, kt16, qt8, kt8, v16, v8 = tiles

        slots = []
        for kt in range(4):  # qb0, bf16, one kt per slot
            q_off = kt * 128
            slots.append(dict(kind=16, qb=0, kt=kt, q_off=q_off, nqf=512 - q_off,
                              first=(kt == 0), last=(kt == 3), diag=True))
        for qb in range(1, 4):  # fp8 pairs
            npair = 2 * qb + 2
            for j in range(npair):
                q_off = 256 if j == npair - 1 else 0
                slots.append(dict(kind=8, qb=qb, j=j, q_off=q_off, nqf=512 - q_off,
                                  first=(j == 0), last=(j == npair - 1),
                                  diag=(j >= 2 * qb)))

        state = {}

        def emit_S(s):
            sp = pool_sp.tile([128, 2, 512], FP32, tag="sp", name="sp")
            nqf = s["nqf"]
            if s["kind"] == 16:
                q0 = s["q_off"]
                for ci in range(2):
                    nc.tensor.matmul(
                        sp[:, 0, :nqf], kt16[:, ci, s["kt"] * 128:(s["kt"] + 1) * 128],
                        qt16[:, ci, q0:q0 + nqf], start=(ci == 0), stop=(ci == 1),
                    )
                p = pool_p16.tile([128, 512], BF, tag="p16", name="p16")
                nc.scalar.activation(p[:, :nqf], sp[:, 0, :nqf],
                                     mybir.ActivationFunctionType.Exp, scale=SCALE)
                nc.vector.tensor_tensor(p[:, :128], p[:, :128], mask_sb[:],
                                        mybir.AluOpType.mult)
            else:
                q0 = s["qb"] * 512 + s["q_off"]
                for i in range(2):
                    kt = 2 * s["j"] + i
                    nc.tensor.matmul(
                        sp[:, i, :nqf], kt8[:, :, kt * 128:(kt + 1) * 128],
                        qt8[:, :, q0:q0 + nqf], start=True, stop=True, perf_mode=DR,
                    )
                p = pool_p8.tile([128, 2, 512], F8, tag="p8", name="p8")
                nc.scalar.activation(p[:, :, :nqf], sp[:, :, :nqf],
                                     mybir.ActivationFunctionType.Exp,
                                     scale=SCALE, bias=shiftb_sb[:])
                if s["diag"]:
                    nc.vector.tensor_tensor(p[:, 0, 0:128], p[:, 0, 0:128],
                                            tri8_sb[:], mybir.AluOpType.mult)
                    nc.vector.tensor_tensor(p[:, 1, 0:256], p[:, 1, 0:256],
                                            ztri8_sb[:], mybir.AluOpType.mult)
            return p

        def emit_PVL(s, p):
            qb, q_off, nqf = s["qb"], s["q_off"], s["nqf"]
            first, last = s["first"], s["last"]
            if first:
                state[qb] = (
                    pool_ot.tile([128, 512], FP32, tag="ot0", name="ot0"),
                    pool_ot.tile([128, 512], FP32, tag="ot1", name="ot1"),
                    pool_l.tile([128, 512], FP32, tag="l", name="lp"),
                )
            ot0, ot1, lp = state[qb]
            if s["kind"] == 16:
                kt = s["kt"]
                nc.tensor.matmul(ot1[:, q_off:], v16[:, kt, 0:128], p[:, :nqf],
                                 start=first, stop=last)
                nc.tensor.matmul(ot0[:, q_off:], v16[:, kt, 128:256], p[:, :nqf],
                                 start=first, stop=last)
                nc.tensor.matmul(lp[:, q_off:], ones16_sb[:],
                                 p[:, :nqf], start=first, stop=last)
            else:
                j2 = 2 * s["j"]
                nc.tensor.matmul(ot1[:, q_off:], v8[:, j2:j2 + 2, 0:128],
                                 p[:, :, :nqf], start=first, stop=last, perf_mode=DR)
                nc.tensor.matmul(ot0[:, q_off:], v8[:, j2:j2 + 2, 128:256],
                                 p[:, :, :nqf], start=first, stop=last, perf_mode=DR)
                nc.tensor.matmul(lp[:, q_off:], ones8_sb[:],
                                 p[:, :, :nqf], start=first, stop=last, perf_mode=DR)
            if last:
                rb = pool_rb.tile([128, 512], FP32, tag="rb", name="rb")
                nc.vector.reciprocal_approx_fast(rb[:], lp[:])
                for ct, otp in ((0, ot1), (1, ot0)):
                    nc.vector.tensor_tensor(
                        ot_sb[:, hp * 2 + ct, qb * 512:(qb + 1) * 512],
                        otp[:], rb[:], mybir.AluOpType.mult,
                    )

        work = []
        emitted = 0
        nslot = len(slots)
        for i, s in enumerate(slots):
            work.append((s, emit_S(s)))
            if i >= 2:
                emit_PVL(*work[i - 2])
            want = (i + 1) * len(next_blocks) // nslot
            while emitted < want:
                next_blocks[emitted]()
                emitted += 1
        emit_PVL(*work[nslot - 2])
        emit_PVL(*work[nslot - 1])
        while emitted < len(next_blocks):
            next_blocks[emitted]()
            emitted += 1

    # --- head 0 QKV with a deep temporary psum ring (attention not started) ---
    h0_ring = {"i": 0}
    h0_tags = [(pool_qp, "qp"), (pool_ot, "ot0"), (pool_ot, "ot1"), (pool_l, "l")]

    def h0_psum():
        pool, tag = h0_tags[h0_ring["i"] % 4]
        h0_ring["i"] += 1
        return pool.tile([128, 512], FP32, tag=tag, name="h0qkv")

    def drip_psum():
        return pool_qp.tile([128, 512], FP32, tag="qp", name="qkvps")

    head_tiles = alloc_head_tiles()
    for b in qkv_blocks(0, head_tiles, h0_psum):
        b()
    for hp in range(HPC):
        if hp + 1 < HPC:
            nxt_tiles = alloc_head_tiles()
            nxt = qkv_blocks(hp + 1, nxt_tiles, drip_psum)
        else:
            nxt_tiles, nxt = None, []
        attention(hp, head_tiles, nxt)
        head_tiles = nxt_tiles

    # --- output projection, transposed: out^T[f, n] = sum_t W[t]^T O^T[t] ---
    prj_tags = [(pool_ot, "ot0"), (pool_ot, "ot1"), (pool_l, "l"), (pool_qp, "qp")]
    idx = 0
    for f in range(2):
        for nch in range(4):
            pool, tag = prj_tags[idx % 4]
            idx += 1
            ps = pool.tile([128, 512], FP32, tag=tag, name="ps_prj")
            for t in range(2 * HPC):
                nc.tensor.matmul(
                    ps[:], wproj_sb[:, t, f * 128:(f + 1) * 128],
                    ot_sb[:, t, nch * 512:(nch + 1) * 512],
                    start=(t == 0), stop=(t == 2 * HPC - 1),
                )
            osb = pool_osb.tile([128, 512], FP32, tag="osb", name="osb")
            nc.vector.tensor_scalar_add(osb[:], ps[:], bias_sb[:, f:f + 1])
            nc.sync.dma_start(out_r[:, f, nch * 512:(nch + 1) * 512], osb[:])


def build_program(nq=N):
    nc = bacc.Bacc(trn_type="TRN2")
    xt_d = nc.dram_tensor("xt", (C, nq), BF, kind="ExternalInput").ap()
    wqkv_d = nc.dram_tensor("wqkv", (C, 3 * HPC * C), BF, kind="ExternalInput").ap()
    wproj_d = nc.dram_tensor("wproj", (2 * HPC * 128, C), BF, kind="ExternalInput").ap()
    bias_d = nc.dram_tensor("bias", (128, 2), mybir.dt.float32, kind="ExternalInput").ap()
    mask_d = nc.dram_tensor("mask", (128, 128), BF, kind="ExternalInput").ap()
    ztri_d = nc.dram_tensor("ztri", (128, 256), BF, kind="ExternalInput").ap()
    out_d = nc.dram_tensor("out", (2 * 128, nq), mybir.dt.float32, kind="ExternalOutput").ap()
    with tile.TileContext(nc) as tc:
        import contextlib
        tc._es = contextlib.ExitStack()
        with tc._es:
            _emit(tc, nq, (xt_d, wqkv_d, wproj_d, bias_d, mask_d, ztri_d, out_d))
    nc.compile()
    return nc


def core_inputs(core, x, w_qkv, w_proj, b_proj, nq=N):
    b, hg = core // 2, core % 2
    heads = list(range(hg * HPC, hg * HPC + HPC))
    xt = np.ascontiguousarray(x[b].T).astype(BF16)
    wr = np.asarray(w_qkv, np.float32).reshape(C, 3, H, C)
    w4 = np.ascontiguousarray(
        wr[:, :, heads, :].transpose(0, 2, 1, 3)
    ).reshape(C, 3 * HPC * C).astype(BF16)
    wp = np.asarray(w_proj, np.float32).reshape(H, C, C)[heads].reshape(HPC * C, C).astype(BF16)
    bias_full = (np.asarray(b_proj, np.float32) if hg == 0
                 else np.zeros(C, np.float32))
    bias2 = np.ascontiguousarray(bias_full.reshape(2, 128).T)  # [p, f]
    tri = (np.arange(128)[:, None] <= np.arange(128)[None, :])
    mask = tri.astype(BF16)
    ztri = np.concatenate([np.zeros((128, 128), bool), tri], axis=1).astype(BF16)
    return {"xt": xt, "wqkv": w4, "wproj": wp, "bias": bias2,
            "mask": mask, "ztri": ztri}


_CACHE = {}


def kernel(x, w_qkv, w_proj, b_proj, **run_kwargs):
    x = np.asarray(x, np.float32)
    w_qkv = np.asarray(w_qkv, np.float32)
    w_proj = np.asarray(w_proj, np.float32)
    b_proj = np.asarray(b_proj, np.float32)
    if "nc" not in _CACHE:
        _CACHE["nc"] = build_program(N)
    nc = _CACHE["nc"]
    in_maps = [core_inputs(c, x, w_qkv, w_proj, b_proj) for c in range(8)]
    res = run_bass_kernel_spmd(nc, in_maps, core_ids=list(range(8)), **run_kwargs)
    out = np.zeros((B, N, C), np.float32)
    for c in range(8):
        out[c // 2] += res.results[c]["out"].T
    _CACHE["last_results"] = res
    return out


# revision 3
# speedup vs baseline: 1.2637x; 1.0669x over previous
"""Causal multi-head attention block (QKV proj + flash-style attention + out proj)
for Trainium2, sharded over 8 NeuronCores as (batch, head-group):
core c -> batch b = c//2, heads hg*4..hg*4+4 with hg = c%2.

Row-split precision hybrid per 512-wide q-block:
  qb0 (rows 0:512, concentrated softmax): bf16 everywhere (error-sensitive).
  qb1-3 (rows 512:2048): fp8(e4m3) DoubleRow matmuls for S, P@V and the
    row-sum L; exp is computed as exp(S*SCALE - SHIFT) so P <= ~60 stays
    inside TRN e4m3 range (+-240); the shift cancels in O/l exactly.
  Row sums l are computed with an all-ones [128,k] stationary so the PE
  broadcasts l to all 128 partitions (no gpsimd partition-broadcast needed).
  Out projection runs transposed (w_proj stationary, O^T moving) for
  stationary reuse; host transposes the (C, N) result back.
"""

import numpy as np
import ml_dtypes

import concourse.bass as bass
import concourse.tile as tile
from concourse import bacc, mybir
from concourse.bass_utils import run_bass_kernel_spmd

B, N, C, H = 4, 2048, 256, 8
SCALE = C ** -0.5
SHIFT = 2.0
BF16 = ml_dtypes.bfloat16
FP32 = mybir.dt.float32
BF = mybir.dt.bfloat16
F8 = mybir.dt.float8e4
HPC = 4  # heads per core
DR = mybir.MatmulPerfMode.DoubleRow


def _emit(tc, nq, aps):
    nc = tc.nc
    nt = nq // 128

    xt_d, wqkv_d, wproj_d, bias_d, mask_d, ztri_d, out_d = aps
    xt_r = xt_d.rearrange("(c p) n -> p c n", p=128)
    wqkv_r = wqkv_d.rearrange("(c p) m -> p c m", p=128)
    wproj_r = wproj_d.rearrange("(t p) f -> p t f", p=128)
    out_r = out_d.rearrange("(t p) n -> p t n", p=128)

    singles = tc._es.enter_context(tc.tile_pool(name="singles", bufs=1))
    pool_head = tc._es.enter_context(tc.tile_pool(name="headp", bufs=2))
    pool_p16 = tc._es.enter_context(tc.tile_pool(name="p16p", bufs=3))
    pool_p8 = tc._es.enter_context(tc.tile_pool(name="p8p", bufs=3))
    pool_rb = tc._es.enter_context(tc.tile_pool(name="rbp", bufs=2))
    pool_osb = tc._es.enter_context(tc.tile_pool(name="osbp", bufs=4))
    # PSUM: sp 2x2 banks + ot0/ot1 2 + l 1 + qp 1 = 8 banks
    pool_sp = tc._es.enter_context(tc.tile_pool(name="psumsp", bufs=2, space="PSUM"))
    pool_ot = tc._es.enter_context(tc.tile_pool(name="psumot", bufs=1, space="PSUM"))
    pool_l = tc._es.enter_context(tc.tile_pool(name="psuml", bufs=1, space="PSUM"))
    pool_qp = tc._es.enter_context(tc.tile_pool(name="psumqp", bufs=1, space="PSUM"))

    # --- SBUF constants / inputs ---
    xt_sb = singles.tile([128, 2, nq], BF)
    wqkv_sb = singles.tile([128, 2, 3 * HPC * C], BF)
    wproj_sb = singles.tile([128, 2 * HPC, C], BF)
    bias_sb = singles.tile([128, 2], FP32)
    mask_sb = singles.tile([128, 128], BF)
    ztri_sb = singles.tile([128, 256], BF)
    tri8_sb = singles.tile([128, 128], F8)
    ztri8_sb = singles.tile([128, 256], F8)
    ones16_sb = singles.tile([128, 128], BF)
    ones8_sb = singles.tile([128, 2, 128], F8)
    shiftb_sb = singles.tile([128, 1], FP32)
    ot_sb = singles.tile([128, 2 * HPC, nq], BF)

    # input DMAs split across both HW-DGE rings
    for ib in range(4):
        nc.sync.dma_start(xt_sb[:, :, ib * 512:(ib + 1) * 512],
                          xt_r[:, :, ib * 512:(ib + 1) * 512])
    for hw in range(HPC):
        c0 = hw * 3 * C
        nc.scalar.dma_start(wqkv_sb[:, :, c0:c0 + 3 * C], wqkv_r[:, :, c0:c0 + 3 * C])
    nc.sync.dma_start(mask_sb[:], mask_d[:])
    nc.sync.dma_start(ztri_sb[:], ztri_d[:])
    nc.scalar.dma_start(wproj_sb[:], wproj_r)
    nc.scalar.dma_start(bias_sb[:], bias_d[:])
    nc.vector.tensor_copy(tri8_sb[:], mask_sb[:])
    nc.vector.tensor_copy(ztri8_sb[:], ztri_sb[:])
    nc.gpsimd.memset(ones16_sb[:], 1.0)
    nc.gpsimd.memset(ones8_sb[:], 1.0)
    nc.gpsimd.memset(shiftb_sb[:], -SHIFT)

    # warm up the PE HAM clock gate while input DMAs land
    warm_sb = singles.tile([128, 512], BF)
    nc.gpsimd.memset(warm_sb[:], 0.0)
    warm_ps = pool_qp.tile([128, 512], FP32, tag="qp", name="warm_ps")
    for wi in range(30):
        nc.tensor.matmul(warm_ps[:], warm_sb[:, :128], warm_sb[:],
                         start=(wi == 0), stop=(wi == 29))

    def alloc_head_tiles():
        qt16 = pool_head.tile([128, 2, 512], BF, tag="qt16", name="qt16")
        kt16 = pool_head.tile([128, 2, 512], BF, tag="kt16", name="kt16")
        qt8 = pool_head.tile([128, 2, nq], F8, tag="qt8", name="qt8")
        kt8 = pool_head.tile([128, 2, nq], F8, tag="kt8", name="kt8")
        v16 = pool_head.tile([128, 4, C], BF, tag="v16", name="v16")
        v8 = pool_head.tile([128, nt, C], F8, tag="v8", name="v8")
        return qt16, kt16, qt8, kt8, v16, v8

    par = {"i": 0}

    def qkv_blocks(hp, tiles, psum_alloc, act_ok):
        """Per-(128x512)-block closures for head hp's QKV projection.
        psum_alloc() -> a [128, 512] fp32 PSUM tile.  act_ok: may use the
        Scalar engine for casts (only before attention starts; during
        attention ACT must stay exp-only or drip casts stall the PE)."""
        qt16, kt16, qt8, kt8, v16, v8 = tiles
        blocks = []

        def cast(dst, src):
            par["i"] += 1
            if act_ok and par["i"] % 2 == 0:
                nc.scalar.copy(dst, src)
            else:
                nc.vector.tensor_copy(dst, src)

        def qk_block(j, ct, ib):
            def go():
                ps = psum_alloc()
                col0 = (hp * 3 + j) * C + ct * 128
                for ci in range(2):
                    nc.tensor.matmul(
                        ps[:], wqkv_sb[:, ci, col0:col0 + 128],
                        xt_sb[:, ci, ib * 512:(ib + 1) * 512],
                        start=(ci == 0), stop=(ci == 1),
                    )
                tgt8 = qt8 if j == 0 else kt8
                if ib == 0:
                    tgt16 = qt16 if j == 0 else kt16
                    cast(tgt16[:, ct, :], ps[:])
                    if j == 1:
                        cast(tgt8[:, ct, 0:512], ps[:])
                else:
                    cast(tgt8[:, ct, ib * 512:(ib + 1) * 512], ps[:])
            return go

        def v_block(it):
            def go():
                ps = psum_alloc()
                vcol = (hp * 3 + 2) * C
                for ci in range(2):
                    nc.tensor.matmul(
                        ps[:, :C], xt_sb[:, ci, it * 128:(it + 1) * 128],
                        wqkv_sb[:, ci, vcol:vcol + C],
                        start=(ci == 0), stop=(ci == 1),
                    )
                cast(v8[:, it, :], ps[:, :C])
                if it < 4:
                    cast(v16[:, it, :], ps[:, :C])
            return go

        # order: qb0-critical first (q/k ib0, v it0-3), then fp8 operands
        for j in (0, 1):
            for ct in range(2):
                blocks.append((0, qk_block(j, ct, 0)))
        for it in range(4):
            blocks.append((0, v_block(it)))
        for j in (1, 0):
            for ct in range(2):
                for ib in range(1, 4):
                    blocks.append((0, qk_block(j, ct, ib)))
        for it in range(4, nt):
            blocks.append((0, v_block(it)))
        return blocks

    otl_rot = {"i": 0}
    otl_tags = [(pool_ot, "ot0"), (pool_ot, "ot1"), (pool_l, "l")]

    def attention(hp, tiles, next_blocks):
        """Flash attention for head hp: qb0 in bf16, qb1-3 in fp8 DoubleRow.
        next head's QKV blocks (or the out-projection for the last head) are
        drip-fed into the PE stream as (min_slot, closure) pairs."""
        qt16, kt16, qt8, kt8, v16, v8 = tiles

        slots = []
        for kt in range(4):  # qb0, bf16, one kt per slot
            q_off = kt * 128
            slots.append(dict(kind=16, qb=0, kt=kt, q_off=q_off, nqf=512 - q_off,
                              first=(kt == 0), last=(kt == 3), diag=True))
        for qb in range(1, 4):  # fp8 pairs
            npair = 2 * qb + 2
            for j in range(npair):
                q_off = 256 if j == npair - 1 else 0
                slots.append(dict(kind=8, qb=qb, j=j, q_off=q_off, nqf=512 - q_off,
                                  first=(j == 0), last=(j == npair - 1),
                                  diag=(j >= 2 * qb)))

        state = {}

        def otl_tiles():
            """Allocate (ot0, ot1, lp) with per-qb tag rotation so each new
            allocation lands on the earliest-freed PSUM bank."""
            r = otl_rot["i"]
            otl_rot["i"] += 1
            tags = [otl_tags[(r + k) % 3] for k in range(3)]
            # emission order per first slot: ot1 MM, ot0 MM, L MM
            ot1 = tags[0][0].tile([128, 512], FP32, tag=tags[0][1], name="ot1")
            ot0 = tags[1][0].tile([128, 512], FP32, tag=tags[1][1], name="ot0")
            lp = tags[2][0].tile([128, 512], FP32, tag=tags[2][1], name="lp")
            return ot0, ot1, lp

        def emit_S(s):
            sp = pool_sp.tile([128, 2, 512], FP32, tag="sp", name="sp")
            nqf = s["nqf"]
            if s["kind"] == 16:
                q0 = s["q_off"]
                for ci in range(2):
                    nc.tensor.matmul(
                        sp[:, 0, :nqf], kt16[:, ci, s["kt"] * 128:(s["kt"] + 1) * 128],
                        qt16[:, ci, q0:q0 + nqf], start=(ci == 0), stop=(ci == 1),
                    )
                p = pool_p16.tile([128, 512], BF, tag="p16", name="p16")
                nc.scalar.activation(p[:, :nqf], sp[:, 0, :nqf],
                                     mybir.ActivationFunctionType.Exp, scale=SCALE)
                nc.vector.tensor_tensor(p[:, :128], p[:, :128], mask_sb[:],
                                        mybir.AluOpType.mult)
            else:
                q0 = s["qb"] * 512 + s["q_off"]
                for i in range(2):
                    kt = 2 * s["j"] + i
                    nc.tensor.matmul(
                        sp[:, i, :nqf], kt8[:, :, kt * 128:(kt + 1) * 128],
                        qt8[:, :, q0:q0 + nqf], start=True, stop=True, perf_mode=DR,
                    )
                p = pool_p8.tile([128, 2, 512], F8, tag="p8", name="p8")
                nc.scalar.activation(p[:, :, :nqf], sp[:, :, :nqf],
                                     mybir.ActivationFunctionType.Exp,
                                     scale=SCALE, bias=shiftb_sb[:])
                if s["diag"]:
                    nc.vector.tensor_tensor(p[:, 0, 0:128], p[:, 0, 0:128],
                                            tri8_sb[:], mybir.AluOpType.mult)
                    nc.vector.tensor_tensor(p[:, 1, 0:256], p[:, 1, 0:256],
                                            ztri8_sb[:], mybir.AluOpType.mult)
            return p

        def emit_PVL(s, p):
            qb, q_off, nqf = s["qb"], s["q_off"], s["nqf"]
            first, last = s["first"], s["last"]
            if first:
                state[qb] = otl_tiles()
            ot0, ot1, lp = state[qb]
            if s["kind"] == 16:
                kt = s["kt"]
                nc.tensor.matmul(ot1[:, q_off:], v16[:, kt, 0:128], p[:, :nqf],
                                 start=first, stop=last)
                nc.tensor.matmul(ot0[:, q_off:], v16[:, kt, 128:256], p[:, :nqf],
                                 start=first, stop=last)
                nc.tensor.matmul(lp[:, q_off:], ones16_sb[:],
                                 p[:, :nqf], start=first, stop=last)
            else:
                j2 = 2 * s["j"]
                nc.tensor.matmul(ot1[:, q_off:], v8[:, j2:j2 + 2, 0:128],
                                 p[:, :, :nqf], start=first, stop=last, perf_mode=DR)
                nc.tensor.matmul(ot0[:, q_off:], v8[:, j2:j2 + 2, 128:256],
                                 p[:, :, :nqf], start=first, stop=last, perf_mode=DR)
                nc.tensor.matmul(lp[:, q_off:], ones8_sb[:],
                                 p[:, :, :nqf], start=first, stop=last, perf_mode=DR)
            if last:
                rb = pool_rb.tile([128, 512], FP32, tag="rb", name="rb")
                nc.vector.reciprocal_approx_fast(rb[:], lp[:])
                for ct, otp in ((0, ot1), (1, ot0)):
                    nc.vector.tensor_tensor(
                        ot_sb[:, hp * 2 + ct, qb * 512:(qb + 1) * 512],
                        otp[:], rb[:], mybir.AluOpType.mult,
                    )

        work = []
        emitted = 0
        nslot = len(slots)

        def drip(i, want):
            nonlocal emitted
            while (emitted < len(next_blocks) and emitted < want
                   and next_blocks[emitted][0] <= i):
                next_blocks[emitted][1]()
                emitted += 1

        for i, s in enumerate(slots):
            work.append((s, emit_S(s)))
            if i >= 2:
                emit_PVL(*work[i - 2])
            drip(i, (i + 1) * len(next_blocks) // nslot)
        emit_PVL(*work[nslot - 2])
        emit_PVL(*work[nslot - 1])
        drip(10 ** 9, len(next_blocks))

    # --- head 0 QKV with a deep temporary psum ring (attention not started) ---
    h0_ring = {"i": 0}
    h0_tags = [(pool_qp, "qp"), (pool_ot, "ot0"), (pool_ot, "ot1"), (pool_l, "l")]

    def h0_psum():
        pool, tag = h0_tags[h0_ring["i"] % 4]
        h0_ring["i"] += 1
        return pool.tile([128, 512], FP32, tag=tag, name="h0qkv")

    def drip_psum():
        return pool_qp.tile([128, 512], FP32, tag="qp", name="qkvps")

    def proj_units():
        """Output projection out^T[f, n] = sum_t W[t]^T O^T[t] + b, one unit
        per (f-chunk, n-chunk), dripped into head 3's attention as soon as
        head 3's q-block n-chunk has been normalized."""
        units = []

        def unit(f, nch):
            def go():
                ps = pool_qp.tile([128, 512], FP32, tag="qp", name="ps_prj")
                for t in range(2 * HPC):
                    nc.tensor.matmul(
                        ps[:], wproj_sb[:, t, f * 128:(f + 1) * 128],
                        ot_sb[:, t, nch * 512:(nch + 1) * 512],
                        start=(t == 0), stop=(t == 2 * HPC - 1),
                    )
                osb = pool_osb.tile([128, 512], FP32, tag="osb", name="osb")
                nc.vector.tensor_scalar_add(osb[:], ps[:], bias_sb[:, f:f + 1])
                nc.sync.dma_start(out_r[:, f, nch * 512:(nch + 1) * 512], osb[:])
            return go

        mins = {0: 6, 1: 10, 2: 16, 3: 22}
        for nch in range(4):
            for f in range(2):
                units.append((mins[nch], unit(f, nch)))
        return units

    head_tiles = alloc_head_tiles()
    for _, b in qkv_blocks(0, head_tiles, h0_psum, act_ok=True):
        b()
    for hp in range(HPC):
        if hp + 1 < HPC:
            nxt_tiles = alloc_head_tiles()
            nxt = qkv_blocks(hp + 1, nxt_tiles, drip_psum, act_ok=False)
        else:
            nxt_tiles, nxt = None, proj_units()
        attention(hp, head_tiles, nxt)
        head_tiles = nxt_tiles


def build_program(nq=N):
    nc = bacc.Bacc(trn_type="TRN2")
    xt_d = nc.dram_tensor("xt", (C, nq), BF, kind="ExternalInput").ap()
    wqkv_d = nc.dram_tensor("wqkv", (C, 3 * HPC * C), BF, kind="ExternalInput").ap()
    wproj_d = nc.dram_tensor("wproj", (2 * HPC * 128, C), BF, kind="ExternalInput").ap()
    bias_d = nc.dram_tensor("bias", (128, 2), mybir.dt.float32, kind="ExternalInput").ap()
    mask_d = nc.dram_tensor("mask", (128, 128), BF, kind="ExternalInput").ap()
    ztri_d = nc.dram_tensor("ztri", (128, 256), BF, kind="ExternalInput").ap()
    out_d = nc.dram_tensor("out", (2 * 128, nq), mybir.dt.float32, kind="ExternalOutput").ap()
    with tile.TileContext(nc) as tc:
        import contextlib
        tc._es = contextlib.ExitStack()
        with tc._es:
            _emit(tc, nq, (xt_d, wqkv_d, wproj_d, bias_d, mask_d, ztri_d, out_d))
    nc.compile()
    return nc


def core_inputs(core, x, w_qkv, w_proj, b_proj, nq=N):
    b, hg = core // 2, core % 2
    heads = list(range(hg * HPC, hg * HPC + HPC))
    xt = np.ascontiguousarray(x[b].T).astype(BF16)
    wr = np.asarray(w_qkv, np.float32).reshape(C, 3, H, C)
    w4 = np.ascontiguousarray(
        wr[:, :, heads, :].transpose(0, 2, 1, 3)
    ).reshape(C, 3 * HPC * C).astype(BF16)
    wp = np.asarray(w_proj, np.float32).reshape(H, C, C)[heads].reshape(HPC * C, C).astype(BF16)
    bias_full = (np.asarray(b_proj, np.float32) if hg == 0
                 else np.zeros(C, np.float32))
    bias2 = np.ascontiguousarray(bias_full.reshape(2, 128).T)  # [p, f]
    tri = (np.arange(128)[:, None] <= np.arange(128)[None, :])
    mask = tri.astype(BF16)
    ztri = np.concatenate([np.zeros((128, 128), bool), tri], axis=1).astype(BF16)
    return {"xt": xt, "wqkv": w4, "wproj": wp, "bias": bias2,
            "mask": mask, "ztri": ztri}


_CACHE = {}


def kernel(x, w_qkv, w_proj, b_proj, **run_kwargs):
    x = np.asarray(x, np.float32)
    w_qkv = np.asarray(w_qkv, np.float32)
    w_proj = np.asarray(w_proj, np.float32)
    b_proj = np.asarray(b_proj, np.float32)
    if "nc" not in _CACHE:
        _CACHE["nc"] = build_program(N)
    nc = _CACHE["nc"]
    in_maps = [core_inputs(c, x, w_qkv, w_proj, b_proj) for c in range(8)]
    res = run_bass_kernel_spmd(nc, in_maps, core_ids=list(range(8)), **run_kwargs)
    out = np.zeros((B, N, C), np.float32)
    for c in range(8):
        out[c // 2] += res.results[c]["out"].T
    _CACHE["last_results"] = res
    return out


# revision 4
# speedup vs baseline: 1.2976x; 1.0268x over previous
"""Causal multi-head attention block (QKV proj + flash-style attention + out proj)
for Trainium2, sharded over 8 NeuronCores as (batch, head-group):
core c -> batch b = c//2, heads hg*4..hg*4+4 with hg = c%2.

Row-split precision hybrid per 512-wide q-block:
  qb0 (rows 0:512, concentrated softmax): bf16 everywhere (error-sensitive).
  qb1-3 (rows 512:2048): fp8(e4m3) DoubleRow matmuls for S, P@V and the
    row-sum L; exp is computed as exp(S*SCALE - SHIFT) so P <= ~60 stays
    inside TRN e4m3 range (+-240); the shift cancels in O/l exactly.
  Row sums l are computed with an all-ones [128,k] stationary so the PE
  broadcasts l to all 128 partitions (no gpsimd partition-broadcast needed).
  Out projection runs transposed (w_proj stationary, O^T moving) for
  stationary reuse; host transposes the (C, N) result back.
"""

import numpy as np
import ml_dtypes

import concourse.bass as bass
import concourse.tile as tile
from concourse import bacc, mybir
from concourse.bass_utils import run_bass_kernel_spmd

B, N, C, H = 4, 2048, 256, 8
SCALE = C ** -0.5
SHIFT = 2.0
BF16 = ml_dtypes.bfloat16
FP32 = mybir.dt.float32
BF = mybir.dt.bfloat16
F8 = mybir.dt.float8e4
HPC = 4  # heads per core
DR = mybir.MatmulPerfMode.DoubleRow


def _emit(tc, nq, aps):
    nc = tc.nc
    nt = nq // 128

    xt_d, wqkv_d, wproj_d, bias_d, mask_d, ztri_d, out_d = aps
    xt_r = xt_d.rearrange("(c p) n -> p c n", p=128)
    wqkv_r = wqkv_d.rearrange("(c p) m -> p c m", p=128)
    wproj_r = wproj_d.rearrange("(t p) f -> p t f", p=128)
    out_r = out_d.rearrange("(t p) n -> p t n", p=128)

    singles = tc._es.enter_context(tc.tile_pool(name="singles", bufs=1))
    pool_head = tc._es.enter_context(tc.tile_pool(name="headp", bufs=2))
    pool_p16 = tc._es.enter_context(tc.tile_pool(name="p16p", bufs=3))
    pool_p8 = tc._es.enter_context(tc.tile_pool(name="p8p", bufs=3))
    pool_rb = tc._es.enter_context(tc.tile_pool(name="rbp", bufs=2))
    pool_osb = tc._es.enter_context(tc.tile_pool(name="osbp", bufs=4))
    # PSUM: sp 2x2 banks + ot0/ot1 2 + l 1 + qp 1 = 8 banks
    pool_sp = tc._es.enter_context(tc.tile_pool(name="psumsp", bufs=2, space="PSUM"))
    pool_ot = tc._es.enter_context(tc.tile_pool(name="psumot", bufs=1, space="PSUM"))
    pool_l = tc._es.enter_context(tc.tile_pool(name="psuml", bufs=1, space="PSUM"))
    pool_qp = tc._es.enter_context(tc.tile_pool(name="psumqp", bufs=1, space="PSUM"))

    # --- SBUF constants / inputs ---
    xt_sb = singles.tile([128, 2, nq], BF)
    wqkv_sb = singles.tile([128, 2, 3 * HPC * C], BF)
    xt8_sb = singles.tile([128, 2, nq], F8)
    wqkv8_sb = singles.tile([128, 2, 2 * HPC * C], F8)  # q,k cols only
    wproj_sb = singles.tile([128, 2 * HPC, C], BF)
    bias_sb = singles.tile([128, 2], FP32)
    mask_sb = singles.tile([128, 128], BF)
    ztri_sb = singles.tile([128, 256], BF)
    tri8_sb = singles.tile([128, 128], F8)
    ztri8_sb = singles.tile([128, 256], F8)
    ones16_sb = singles.tile([128, 128], BF)
    ones8_sb = singles.tile([128, 2, 128], F8)
    shiftb_sb = singles.tile([128, 1], FP32)
    ot_sb = singles.tile([128, 2 * HPC, nq], BF)

    # input DMAs split across both HW-DGE rings
    for ib in range(4):
        nc.sync.dma_start(xt_sb[:, :, ib * 512:(ib + 1) * 512],
                          xt_r[:, :, ib * 512:(ib + 1) * 512])
    for hw in range(HPC):
        c0 = hw * 3 * C
        nc.scalar.dma_start(wqkv_sb[:, :, c0:c0 + 3 * C], wqkv_r[:, :, c0:c0 + 3 * C])
    nc.sync.dma_start(mask_sb[:], mask_d[:])
    nc.sync.dma_start(ztri_sb[:], ztri_d[:])
    nc.scalar.dma_start(wproj_sb[:], wproj_r)
    nc.scalar.dma_start(bias_sb[:], bias_d[:])
    nc.vector.tensor_copy(tri8_sb[:], mask_sb[:])
    nc.vector.tensor_copy(ztri8_sb[:], ztri_sb[:])
    for ib in range(4):
        nc.vector.tensor_copy(xt8_sb[:, :, ib * 512:(ib + 1) * 512],
                              xt_sb[:, :, ib * 512:(ib + 1) * 512])
    for hw in range(HPC):
        nc.vector.tensor_copy(wqkv8_sb[:, :, hw * 512:(hw + 1) * 512],
                              wqkv_sb[:, :, hw * 3 * C:hw * 3 * C + 2 * C])
    nc.gpsimd.memset(ones16_sb[:], 1.0)
    nc.gpsimd.memset(ones8_sb[:], 1.0)
    nc.gpsimd.memset(shiftb_sb[:], -SHIFT)

    # warm up the PE HAM clock gate while input DMAs land
    warm_sb = singles.tile([128, 512], BF)
    nc.gpsimd.memset(warm_sb[:], 0.0)
    warm_ps = pool_qp.tile([128, 512], FP32, tag="qp", name="warm_ps")
    for wi in range(30):
        nc.tensor.matmul(warm_ps[:], warm_sb[:, :128], warm_sb[:],
                         start=(wi == 0), stop=(wi == 29))

    def alloc_head_tiles():
        qt16 = pool_head.tile([128, 2, 512], BF, tag="qt16", name="qt16")
        kt16 = pool_head.tile([128, 2, 512], BF, tag="kt16", name="kt16")
        qt8 = pool_head.tile([128, 2, nq], F8, tag="qt8", name="qt8")
        kt8 = pool_head.tile([128, 2, nq], F8, tag="kt8", name="kt8")
        v16 = pool_head.tile([128, 4, C], BF, tag="v16", name="v16")
        v8 = pool_head.tile([128, nt, C], F8, tag="v8", name="v8")
        return qt16, kt16, qt8, kt8, v16, v8

    par = {"i": 0}

    def qkv_blocks(hp, tiles, psum_alloc, act_ok):
        """Per-(128x512)-block closures for head hp's QKV projection.
        psum_alloc() -> a [128, 512] fp32 PSUM tile.  act_ok: may use the
        Scalar engine for casts (only before attention starts; during
        attention ACT must stay exp-only or drip casts stall the PE)."""
        qt16, kt16, qt8, kt8, v16, v8 = tiles
        blocks = []

        def cast(dst, src):
            par["i"] += 1
            if act_ok and par["i"] % 2 == 0:
                nc.scalar.copy(dst, src)
            else:
                nc.vector.tensor_copy(dst, src)

        def qk_block(j, ct, ib):
            def go():
                ps = psum_alloc()
                tgt8 = qt8 if j == 0 else kt8
                if ib == 0:
                    col0 = (hp * 3 + j) * C + ct * 128
                    for ci in range(2):
                        nc.tensor.matmul(
                            ps[:], wqkv_sb[:, ci, col0:col0 + 128],
                            xt_sb[:, ci, ib * 512:(ib + 1) * 512],
                            start=(ci == 0), stop=(ci == 1),
                        )
                    tgt16 = qt16 if j == 0 else kt16
                    cast(tgt16[:, ct, :], ps[:])
                    if j == 1:
                        cast(tgt8[:, ct, 0:512], ps[:])
                else:
                    col8 = (hp * 2 + j) * C + ct * 128
                    nc.tensor.matmul(
                        ps[:], wqkv8_sb[:, :, col8:col8 + 128],
                        xt8_sb[:, :, ib * 512:(ib + 1) * 512],
                        start=True, stop=True, perf_mode=DR,
                    )
                    cast(tgt8[:, ct, ib * 512:(ib + 1) * 512], ps[:])
            return go

        def v_block(it):
            def go():
                ps = psum_alloc()
                vcol = (hp * 3 + 2) * C
                for ci in range(2):
                    nc.tensor.matmul(
                        ps[:, :C], xt_sb[:, ci, it * 128:(it + 1) * 128],
                        wqkv_sb[:, ci, vcol:vcol + C],
                        start=(ci == 0), stop=(ci == 1),
                    )
                cast(v8[:, it, :], ps[:, :C])
                if it < 4:
                    cast(v16[:, it, :], ps[:, :C])
            return go

        # order: qb0-critical first (q/k ib0, v it0-3), then fp8 operands
        for j in (0, 1):
            for ct in range(2):
                blocks.append((0, qk_block(j, ct, 0)))
        for it in range(4):
            blocks.append((0, v_block(it)))
        for j in (1, 0):
            for ct in range(2):
                for ib in range(1, 4):
                    blocks.append((0, qk_block(j, ct, ib)))
        for it in range(4, nt):
            blocks.append((0, v_block(it)))
        return blocks

    otl_rot = {"i": 0}
    otl_tags = [(pool_ot, "ot0"), (pool_ot, "ot1"), (pool_l, "l")]

    def attention(hp, tiles, next_blocks):
        """Flash attention for head hp: qb0 in bf16, qb1-3 in fp8 DoubleRow.
        next head's QKV blocks (or the out-projection for the last head) are
        drip-fed into the PE stream as (min_slot, closure) pairs."""
        qt16, kt16, qt8, kt8, v16, v8 = tiles

        slots = []
        for kt in range(4):  # qb0, bf16, one kt per slot
            q_off = kt * 128
            slots.append(dict(kind=16, qb=0, kt=kt, q_off=q_off, nqf=512 - q_off,
                              first=(kt == 0), last=(kt == 3), diag=True))
        for qb in range(1, 4):  # fp8 pairs
            npair = 2 * qb + 2
            for j in range(npair):
                q_off = 256 if j == npair - 1 else 0
                slots.append(dict(kind=8, qb=qb, j=j, q_off=q_off, nqf=512 - q_off,
                                  first=(j == 0), last=(j == npair - 1),
                                  diag=(j >= 2 * qb)))

        state = {}

        def otl_tiles():
            """Allocate (ot0, ot1, lp) with per-qb tag rotation so each new
            allocation lands on the earliest-freed PSUM bank."""
            r = otl_rot["i"]
            otl_rot["i"] += 1
            tags = [otl_tags[(r + k) % 3] for k in range(3)]
            # emission order per first slot: ot1 MM, ot0 MM, L MM
            ot1 = tags[0][0].tile([128, 512], FP32, tag=tags[0][1], name="ot1")
            ot0 = tags[1][0].tile([128, 512], FP32, tag=tags[1][1], name="ot0")
            lp = tags[2][0].tile([128, 512], FP32, tag=tags[2][1], name="lp")
            return ot0, ot1, lp

        def emit_S(s):
            sp = pool_sp.tile([128, 2, 512], FP32, tag="sp", name="sp")
            nqf = s["nqf"]
            if s["kind"] == 16:
                q0 = s["q_off"]
                for ci in range(2):
                    nc.tensor.matmul(
                        sp[:, 0, :nqf], kt16[:, ci, s["kt"] * 128:(s["kt"] + 1) * 128],
                        qt16[:, ci, q0:q0 + nqf], start=(ci == 0), stop=(ci == 1),
                    )
                p = pool_p16.tile([128, 512], BF, tag="p16", name="p16")
                nc.scalar.activation(p[:, :nqf], sp[:, 0, :nqf],
                                     mybir.ActivationFunctionType.Exp, scale=SCALE)
                nc.vector.tensor_tensor(p[:, :128], p[:, :128], mask_sb[:],
                                        mybir.AluOpType.mult)
            else:
                q0 = s["qb"] * 512 + s["q_off"]
                for i in range(2):
                    kt = 2 * s["j"] + i
                    nc.tensor.matmul(
                        sp[:, i, :nqf], kt8[:, :, kt * 128:(kt + 1) * 128],
                        qt8[:, :, q0:q0 + nqf], start=True, stop=True, perf_mode=DR,
                    )
                p = pool_p8.tile([128, 2, 512], F8, tag="p8", name="p8")
                nc.scalar.activation(p[:, :, :nqf], sp[:, :, :nqf],
                                     mybir.ActivationFunctionType.Exp,
                                     scale=SCALE, bias=shiftb_sb[:])
                if s["diag"]:
                    nc.vector.tensor_tensor(p[:, 0, 0:128], p[:, 0, 0:128],
                                            tri8_sb[:], mybir.AluOpType.mult)
                    nc.vector.tensor_tensor(p[:, 1, 0:256], p[:, 1, 0:256],
                                            ztri8_sb[:], mybir.AluOpType.mult)
            return p

        def emit_PVL(s, p):
            qb, q_off, nqf = s["qb"], s["q_off"], s["nqf"]
            first, last = s["first"], s["last"]
            if first:
                state[qb] = otl_tiles()
            ot0, ot1, lp = state[qb]
            if s["kind"] == 16:
                kt = s["kt"]
                nc.tensor.matmul(ot1[:, q_off:], v16[:, kt, 0:128], p[:, :nqf],
                                 start=first, stop=last)
                nc.tensor.matmul(ot0[:, q_off:], v16[:, kt, 128:256], p[:, :nqf],
                                 start=first, stop=last)
                nc.tensor.matmul(lp[:, q_off:], ones16_sb[:],
                                 p[:, :nqf], start=first, stop=last)
            else:
                j2 = 2 * s["j"]
                nc.tensor.matmul(ot1[:, q_off:], v8[:, j2:j2 + 2, 0:128],
                                 p[:, :, :nqf], start=first, stop=last, perf_mode=DR)
                nc.tensor.matmul(ot0[:, q_off:], v8[:, j2:j2 + 2, 128:256],
                                 p[:, :, :nqf], start=first, stop=last, perf_mode=DR)
                nc.tensor.matmul(lp[:, q_off:], ones8_sb[:],
                                 p[:, :, :nqf], start=first, stop=last, perf_mode=DR)
            if last:
                rb = pool_rb.tile([128, 512], FP32, tag="rb", name="rb")
                nc.vector.reciprocal_approx_fast(rb[:], lp[:])
                for ct, otp in ((0, ot1), (1, ot0)):
                    nc.vector.tensor_tensor(
                        ot_sb[:, hp * 2 + ct, qb * 512:(qb + 1) * 512],
                        otp[:], rb[:], mybir.AluOpType.mult,
                    )

        work = []
        emitted = 0
        nslot = len(slots)

        def drip(i, want):
            nonlocal emitted
            while (emitted < len(next_blocks) and emitted < want
                   and next_blocks[emitted][0] <= i):
                next_blocks[emitted][1]()
                emitted += 1

        for i, s in enumerate(slots):
            work.append((s, emit_S(s)))
            if i >= 2:
                emit_PVL(*work[i - 2])
            drip(i, (i + 1) * len(next_blocks) // nslot)
        emit_PVL(*work[nslot - 2])
        emit_PVL(*work[nslot - 1])
        drip(10 ** 9, len(next_blocks))

    # --- head 0 QKV with a deep temporary psum ring (attention not started) ---
    h0_ring = {"i": 0}
    h0_tags = [(pool_qp, "qp"), (pool_ot, "ot0"), (pool_ot, "ot1"), (pool_l, "l")]

    def h0_psum():
        pool, tag = h0_tags[h0_ring["i"] % 4]
        h0_ring["i"] += 1
        return pool.tile([128, 512], FP32, tag=tag, name="h0qkv")

    def drip_psum():
        return pool_qp.tile([128, 512], FP32, tag="qp", name="qkvps")

    def proj_units():
        """Output projection out^T[f, n] = sum_t W[t]^T O^T[t] + b, one unit
        per (f-chunk, n-chunk), dripped into head 3's attention as soon as
        head 3's q-block n-chunk has been normalized."""
        units = []

        def unit(f, nch):
            def go():
                ps = pool_qp.tile([128, 512], FP32, tag="qp", name="ps_prj")
                for t in range(2 * HPC):
                    nc.tensor.matmul(
                        ps[:], wproj_sb[:, t, f * 128:(f + 1) * 128],
                        ot_sb[:, t, nch * 512:(nch + 1) * 512],
                        start=(t == 0), stop=(t == 2 * HPC - 1),
                    )
                osb = pool_osb.tile([128, 512], FP32, tag="osb", name="osb")
                nc.vector.tensor_scalar_add(osb[:], ps[:], bias_sb[:, f:f + 1])
                nc.sync.dma_start(out_r[:, f, nch * 512:(nch + 1) * 512], osb[:])
            return go

        mins = {0: 6, 1: 10, 2: 16, 3: 22}
        for nch in range(4):
            for f in range(2):
                units.append((mins[nch], unit(f, nch)))
        return units

    head_tiles = alloc_head_tiles()
    for _, b in qkv_blocks(0, head_tiles, h0_psum, act_ok=True):
        b()
    for hp in range(HPC):
        if hp + 1 < HPC:
            nxt_tiles = alloc_head_tiles()
            nxt = qkv_blocks(hp + 1, nxt_tiles, drip_psum, act_ok=False)
        else:
            nxt_tiles, nxt = None, proj_units()
        attention(hp, head_tiles, nxt)
        head_tiles = nxt_tiles


def build_program(nq=N):
    nc = bacc.Bacc(trn_type="TRN2")
    xt_d = nc.dram_tensor("xt", (C, nq), BF, kind="ExternalInput").ap()
    wqkv_d = nc.dram_tensor("wqkv", (C, 3 * HPC * C), BF, kind="ExternalInput").ap()
    wproj_d = nc.dram_tensor("wproj", (2 * HPC * 128, C), BF, kind="ExternalInput").ap()
    bias_d = nc.dram_tensor("bias", (128, 2), mybir.dt.float32, kind="ExternalInput").ap()
    mask_d = nc.dram_tensor("mask", (128, 128), BF, kind="ExternalInput").ap()
    ztri_d = nc.dram_tensor("ztri", (128, 256), BF, kind="ExternalInput").ap()
    out_d = nc.dram_tensor("out", (2 * 128, nq), mybir.dt.float32, kind="ExternalOutput").ap()
    with tile.TileContext(nc) as tc:
        import contextlib
        tc._es = contextlib.ExitStack()
        with tc._es:
            _emit(tc, nq, (xt_d, wqkv_d, wproj_d, bias_d, mask_d, ztri_d, out_d))
    nc.compile()
    return nc


def core_inputs(core, x, w_qkv, w_proj, b_proj, nq=N):
    b, hg = core // 2, core % 2
    heads = list(range(hg * HPC, hg * HPC + HPC))
    xt = np.ascontiguousarray(x[b].T).astype(BF16)
    wr = np.asarray(w_qkv, np.float32).reshape(C, 3, H, C)
    w4 = np.ascontiguousarray(
        wr[:, :, heads, :].transpose(0, 2, 1, 3)
    ).reshape(C, 3 * HPC * C).astype(BF16)
    wp = np.asarray(w_proj, np.float32).reshape(H, C, C)[heads].reshape(HPC * C, C).astype(BF16)
    bias_full = (np.asarray(b_proj, np.float32) if hg == 0
                 else np.zeros(C, np.float32))
    bias2 = np.ascontiguousarray(bias_full.reshape(2, 128).T)  # [p, f]
    tri = (np.arange(128)[:, None] <= np.arange(128)[None, :])
    mask = tri.astype(BF16)
    ztri = np.concatenate([np.zeros((128, 128), bool), tri], axis=1).astype(BF16)
    return {"xt": xt, "wqkv": w4, "wproj": wp, "bias": bias2,
            "mask": mask, "ztri": ztri}


_CACHE = {}


def kernel(x, w_qkv, w_proj, b_proj, **run_kwargs):
    x = np.asarray(x, np.float32)
    w_qkv = np.asarray(w_qkv, np.float32)
    w_proj = np.asarray(w_proj, np.float32)
    b_proj = np.asarray(b_proj, np.float32)
    if "nc" not in _CACHE:
        _CACHE["nc"] = build_program(N)
    nc = _CACHE["nc"]
    in_maps = [core_inputs(c, x, w_qkv, w_proj, b_proj) for c in range(8)]
    res = run_bass_kernel_spmd(nc, in_maps, core_ids=list(range(8)), **run_kwargs)
    out = np.zeros((B, N, C), np.float32)
    for c in range(8):
        out[c // 2] += res.results[c]["out"].T
    _CACHE["last_results"] = res
    return out
